# revision 16
# baseline (speedup 1.0000x reference)
"""Distributed Bass kernel for 2-layer AntiSymmetricConv GNN on 8 TRN2 NeuronCores.

Strategy (graph/data parallel, dst-sharded):
  - Nodes sharded across 8 cores (6250 each, padded to 6272 = 49 chunks of 128).
  - Per layer: each core computes hs = (y @ lin^T) * dinv for its shard (PE),
    AllGathers the bf16 table (Shared addr space) to every core's DRAM.
  - Edges are assigned to the core owning dst. Messages hs[src] are fetched with
    SWDGE dma_gather (arbitrary-index row gather), round-robined over 4 SWDGE
    queues for drain parallelism.
  - Scatter-add runs on the TensorEngine: per 128-edge group a span-packed
    one-hot matrix D (host-built, streamed from DRAM per window, entries
    dinv[dst]) is the stationary operand; the gathered msg tile is the moving
    operand; PSUM accumulates [dst-node x feat] chunks.  Self-loop term enters
    as a host-built diag(dinv) stationary; bias as a K=1 outer-product matmul;
    the antisymmetric term x @ aw^T as an x^T-chunk stationary matmul.
  - Epilogue (tanh/residual/leaky-relu, final softmax) on ACT + DVE at window
    granularity.
All index preprocessing (degrees, edge->core/chunk assignment, gather indices,
one-hot construction) happens on the host from edge_index only.
"""

import os
import sys

sys.path.insert(0, "/opt/trn_rl_repo")

import numpy as np
import ml_dtypes

BF16 = ml_dtypes.bfloat16
EPS = 0.1
GAMMA = 0.1

LAST_EXEC_NS = {"ns": None}


def _ceil(a, b):
    return -(-a // b)


# ---------------------------------------------------------------------------
# Host-side preprocessing (index-only work + weight re-layout)
# ---------------------------------------------------------------------------


class Prep:
    pass


def _host_prep(edge_index, N, n_cores, wchunks=4):
    """Build the shared (core-independent) schedule and per-core index data.

    Subsets are src partition-halves (q<64 / q>=64): each subset has its own
    half-table (AllGather of 64 hs partitions), so the two AGs per layer can
    overlap with the other subset's gathers.  Gather padding happens at
    (window, subset) granularity; 128-edge groups may straddle chunk
    boundaries, yielding one D pair per (group, chunk) combination.
    """
    p = Prep()
    S = N // n_cores
    assert S * n_cores == N
    SPAD = _ceil(S, 128) * 128
    NCH = SPAD // 128
    HROWS = 64 * NCH           # rows per rank per half-table
    THROWS = n_cores * HROWS   # half-table rows
    assert THROWS < 32768

    src = np.asarray(edge_index[0], dtype=np.int64)
    dst = np.asarray(edge_index[1], dtype=np.int64)
    E = src.shape[0]

    deg = np.bincount(dst, minlength=N).astype(np.float64) + 1.0
    dinv = (1.0 / np.sqrt(deg)).astype(np.float32)

    core = dst // S
    dloc = dst - core * S
    chunk = dloc // 128
    sl = src % S
    sc = src // S
    sq = sl % 128
    sub = sq // 64
    trow = sc * HROWS + (sq - 64 * sub) * NCH + (sl // 128)

    windows = [(a, min(a + wchunks, NCH)) for a in range(0, NCH, wchunks)]
    wof = np.zeros(NCH, np.int64)   # window index per chunk
    for wi, (w0, w1) in enumerate(windows):
        wof[w0:w1] = wi
    ewin = wof[chunk]

    NW = len(windows)

    # slot of each edge's source row within its (core, window, sub) gather
    # stream: rows deduped per stream (multiple edges may share one gathered
    # row), ordered by (chunk of first use, trow)
    flat = (core * NW + ewin) * 2 + sub
    order = np.lexsort((trow, chunk, flat))
    sf = flat[order]
    st = trow[order]
    new_seg = np.concatenate([[True], sf[1:] != sf[:-1]])
    new_row = new_seg | np.concatenate([[True], st[1:] != st[:-1]])
    rix = np.cumsum(new_row) - 1              # global unique-row id
    segfirst_rix = np.zeros(n_cores * NW * 2, np.int64)
    segfirst_rix[sf[new_seg]] = rix[new_seg]
    pos_sorted = rix - segfirst_rix[sf]       # slot within stream
    pos = np.empty(E, np.int64)
    pos[order] = pos_sorted
    segcnt = np.zeros(n_cores * NW * 2, np.int64)
    np.maximum.at(segcnt, flat, pos + 1)
    segcnt = segcnt.reshape(n_cores, NW, 2)

    Lws = 128 * np.maximum(_ceil(segcnt.max(axis=0), 128), 1)  # [NW, 2]

    e_group = pos // 128
    e_pp = pos % 128
    e_col = dloc - chunk * 128

    # pair = (window, sub, group, chunk); build union over cores
    gmax = int(Lws.max()) // 128
    pair_of = {}
    pair_list = []  # (wi, s, g, c)
    seen = np.zeros((NW, 2, gmax, wchunks), bool)
    np.logical_or.at(seen, (ewin, sub, e_group, chunk - np.array([w[0] for w in windows])[ewin]), True)
    for wi in range(NW):
        w0, w1 = windows[wi]
        for c in range(w0, w1):
            for s in (0, 1):
                for g in range(Lws[wi, s] // 128):
                    if seen[wi, s, g, c - w0]:
                        pair_of[(wi, s, g, c)] = len(pair_list)
                        pair_list.append((wi, s, g, c))
    npair = len(pair_list)

    # vectorized pair id lookup
    pid_lut = np.full((NW, 2, gmax, wchunks), -1, np.int64)
    for idx_, (wi, s, g, c) in enumerate(pair_list):
        pid_lut[wi, s, g, c - windows[wi][0]] = idx_
    e_pair = pid_lut[ewin, sub, e_group,
                     chunk - np.array([w[0] for w in windows])[ewin]]
    assert (e_pair >= 0).all()

    pair_lo = np.full(npair, 128, np.int64)
    pair_hi = np.full(npair, -1, np.int64)
    np.minimum.at(pair_lo, e_pair, e_col)
    np.maximum.at(pair_hi, e_pair, e_col)
    empty = pair_hi < 0
    pair_lo[empty] = 0
    pair_hi[empty] = 0
    # PE matmul output row-group rules: base 0 (<=128 rows), base 32
    # (<=32 rows, i.e. hi<64), base 64 (<=64 rows)
    base = np.zeros(npair, np.int64)
    base[(pair_lo >= 32) & (pair_hi < 64)] = 32
    base[pair_lo >= 64] = 64
    pair_lo = base
    pair_span = pair_hi - pair_lo + 1
    dcoloff = np.concatenate([[0], np.cumsum(pair_span)])
    DCOLS = int(dcoloff[-1])

    # per-window D column range and per-chunk pair emission lists
    wpairrange = []
    chunk_pairs = [[] for _ in range(NCH)]  # (s, g, pi)
    pi0 = 0
    for wi in range(NW):
        w0, w1 = windows[wi]
        lo = npair
        hi = 0
        for c in range(w0, w1):
            for s in (0, 1):
                for g in range(Lws[wi, s] // 128):
                    key = (wi, s, g, c)
                    if key in pair_of:
                        pi = pair_of[key]
                        chunk_pairs[c].append((s, g, pi))
                        lo = min(lo, pi)
                        hi = max(hi, pi + 1)
        wpairrange.append((lo, hi))
    # pairs are emitted in (wi, ...) blocks; verify contiguity per window
    for wi, (lo, hi) in enumerate(wpairrange):
        for idx_ in range(lo, hi):
            assert pair_list[idx_][0] == wi

    # gather calls: one per (window, subset)
    calls = []
    off16 = [0, 0]
    for wi in range(NW):
        w0, w1 = windows[wi]
        for s in (0, 1):
            L = int(Lws[wi, s])
            calls.append(dict(w0=w0, w1=w1, s=s, off16=off16[s], L=L,
                              G=L // 128))
            off16[s] += L // 16
    idx_cols16 = [off16[0], off16[1]]
    segoff = np.zeros((NW, 2), np.int64)
    ci = 0
    for wi in range(NW):
        for s in (0, 1):
            segoff[wi, s] = calls[ci]["off16"] * 16
            ci += 1

    idx_arrs = []
    dall_arrs = []
    for i in range(n_cores):
        m = core == i
        idxs = [np.zeros(idx_cols16[0] * 16, np.int16),
                np.zeros(idx_cols16[1] * 16, np.int16)]
        gpos = segoff[ewin[m], sub[m]] + pos[m]
        tr = trow[m].astype(np.int16)
        for s in (0, 1):
            ms = sub[m] == s
            idxs[s][gpos[ms]] = tr[ms]
        dall = np.zeros((128, DCOLS), np.float32)
        np.add.at(dall, (e_pp[m], dcoloff[e_pair[m]]
                         + (e_col[m] - pair_lo[e_pair[m]])), dinv[dst[m]])
        wrapped = []
        for s in (0, 1):
            a = idxs[s].reshape(-1, 16).T
            wrapped.append(np.tile(a, (8, 1)).astype(np.int16))
        idx_arrs.append(wrapped)
        dall_arrs.append(dall.astype(BF16))

    p.S, p.SPAD, p.NCH = S, SPAD, NCH
    p.HROWS, p.THROWS = HROWS, THROWS
    p.E, p.N, p.n_cores = E, N, n_cores
    p.dinv = dinv
    p.windows, p.calls, p.npair = windows, calls, npair
    p.idx_cols16 = idx_cols16
    p.wpairrange, p.pair_lo, p.pair_span, p.dcoloff, p.DCOLS = (
        wpairrange, pair_lo, pair_span, dcoloff, DCOLS)
    p.chunk_pairs = chunk_pairs
    p.idx_arrs, p.dall_arrs = idx_arrs, dall_arrs
    return p


def _per_core_inputs(p, x, W1, b1, lin1, W2, b2, lin2):
    """Build in_maps (list of dicts) for run_bass_kernel_spmd."""
    F = x.shape[1]
    S, SPAD, NCH = p.S, p.SPAD, p.NCH
    aw1 = W1 - W1.T - GAMMA * np.eye(F, dtype=np.float32)
    aw2 = W2 - W2.T - GAMMA * np.eye(F, dtype=np.float32)
    ident = np.eye(128, dtype=np.float32)
    in_maps = []
    for i in range(p.n_cores):
        xi = np.zeros((SPAD, F), np.float32)
        xi[:S] = x[i * S:(i + 1) * S]
        dinv_i = np.ones(SPAD, np.float32)
        dinv_i[:S] = p.dinv[i * S:(i + 1) * S]
        x_pm = xi.reshape(NCH, 128, F).transpose(1, 0, 2).reshape(128, NCH * F)
        dinv_col = dinv_i.reshape(NCH, 128).T.copy()  # [128, NCH]
        diag = np.zeros((128, NCH, 128), np.float32)
        rng = np.arange(128)
        diag[rng, :, rng] = dinv_col
        in_maps.append({
            "xT": np.ascontiguousarray(xi.T).astype(BF16),
            "xn": x_pm.astype(np.float32),
            "dinvc": dinv_col.astype(np.float32),
            "diag": diag.reshape(128, NCH * 128).astype(BF16),
            "lin1r": lin1.T.astype(BF16), "aw1r": aw1.T.astype(BF16),
            "lin2r": lin2.T.astype(BF16), "aw2r": aw2.T.astype(BF16),
            "b1r": b1.reshape(1, F).astype(BF16),
            "b2r": b2.reshape(1, F).astype(BF16),
            "onesr": np.ones((1, 128), BF16),
            "ident": ident,
            "idxlo": p.idx_arrs[i][0], "idxhi": p.idx_arrs[i][1],
            "dall": p.dall_arrs[i],
        })
    return in_maps


# ---------------------------------------------------------------------------
# Bass graph
# ---------------------------------------------------------------------------


def _build(p):
    import concourse.bass as bass
    import concourse.bacc as bacc
    import concourse.mybir as mybir
    import concourse.tile as tile

    dt = mybir.dt
    F = 128
    SPAD, NCH = p.SPAD, p.NCH
    HROWS, THROWS = p.HROWS, p.THROWS

    nc = bacc.Bacc("TRN2", target_bir_lowering=False, debug=False,
                   num_devices=p.n_cores, num_swdge_queues=4)

    def din(name, shape, d):
        return nc.dram_tensor(name, shape, d, kind="ExternalInput")

    xT_d = din("xT", [F, SPAD], dt.bfloat16)
    xn_d = din("xn", [128, SPAD], dt.float32)
    dinvc_d = din("dinvc", [128, NCH], dt.float32)
    diag_d = din("diag", [128, NCH * 128], dt.bfloat16)
    lin1r_d = din("lin1r", [F, F], dt.bfloat16)
    aw1r_d = din("aw1r", [F, F], dt.bfloat16)
    lin2r_d = din("lin2r", [F, F], dt.bfloat16)
    aw2r_d = din("aw2r", [F, F], dt.bfloat16)
    b1r_d = din("b1r", [1, F], dt.bfloat16)
    b2r_d = din("b2r", [1, F], dt.bfloat16)
    ones_d = din("onesr", [1, 128], dt.bfloat16)
    ident_d = din("ident", [128, 128], dt.float32)
    idxlo_d = din("idxlo", [128, p.idx_cols16[0]], dt.int16)
    idxhi_d = din("idxhi", [128, p.idx_cols16[1]], dt.int16)
    dall_d = din("dall", [128, p.DCOLS], dt.bfloat16)
    out_d = nc.dram_tensor("out", [128, SPAD], dt.float32, kind="ExternalOutput")

    AF = mybir.ActivationFunctionType
    ALU = mybir.AluOpType
    AX = mybir.AxisListType

    with tile.TileContext(nc) as tc:
        with tc.tile_pool(name="const", bufs=1) as cp, \
             tc.tile_pool(name="msg", bufs=3) as mp, \
             tc.tile_pool(name="dwin", bufs=2) as dwp, \
             tc.tile_pool(name="tmp", bufs=2) as tp, \
             tc.tile_pool(name="sc", bufs=3) as scp, \
             tc.tile_pool(name="acc", bufs=4, space="PSUM") as pacc, \
             tc.tile_pool(name="psh", bufs=2, space="PSUM") as ph, \
             tc.tile_pool(name="dram", bufs=1, space="DRAM") as dr:

            def load(d, shape, dtp, tag):
                t = cp.tile(shape, dtp, tag=tag)
                nc.sync.dma_start(t[:], d.ap())
                return t

            xT = load(xT_d, [F, SPAD], dt.bfloat16, "xT")
            xn = load(xn_d, [128, SPAD], dt.float32, "xn")
            dinvc = load(dinvc_d, [128, NCH], dt.float32, "dinvc")
            diag = load(diag_d, [128, NCH * 128], dt.bfloat16, "diag")
            lin1r = load(lin1r_d, [F, F], dt.bfloat16, "lin1r")
            aw1r = load(aw1r_d, [F, F], dt.bfloat16, "aw1r")
            lin2r = load(lin2r_d, [F, F], dt.bfloat16, "lin2r")
            aw2r = load(aw2r_d, [F, F], dt.bfloat16, "aw2r")
            b1r = load(b1r_d, [1, F], dt.bfloat16, "b1r")
            b2r = load(b2r_d, [1, F], dt.bfloat16, "b2r")
            onesr = load(ones_d, [1, 128], dt.bfloat16, "onesr")
            ident = load(ident_d, [128, 128], dt.float32, "ident")
            idx_sb = [load(idxlo_d, [128, p.idx_cols16[0]], dt.int16, "idxlo"),
                      load(idxhi_d, [128, p.idx_cols16[1]], dt.int16, "idxhi")]

            y2n = cp.tile([128, SPAD], dt.float32, tag="y2n")
            y2T = cp.tile([F, SPAD], dt.bfloat16, tag="y2T")
            outsb = None

            rg = [list(range(p.n_cores))]

            for li in range(2):
                statT = xT if li == 0 else y2T
                resid = xn if li == 0 else y2n
                linr = lin1r if li == 0 else lin2r
                awr = aw1r if li == 0 else aw2r
                br = b1r if li == 0 else b2r

                # ---- hs = (y @ lin^T) * dinv  (bf16, node-orient) ----
                hs = cp.tile([128, SPAD], dt.bfloat16, tag="hs")
                for c in range(NCH):
                    cs = slice(c * 128, (c + 1) * 128)
                    hp = ph.tile([128, F], dt.float32, tag="hp")
                    nc.tensor.matmul(hp[:], statT[:, cs], linr[:],
                                     start=True, stop=True)
                    nc.scalar.activation(hs[:, cs], hp[:], AF.Copy,
                                         scale=dinvc[:, c:c + 1])

                # ---- AllGather bf16 half-tables ----
                tables = []
                for s_ in (0, 1):
                    bounce = dr.tile([HROWS, F], dt.bfloat16,
                                     tag=f"bounce{li}_{s_}")
                    nc.sync.dma_start(
                        bounce[:, :].rearrange("(q c) f -> q (c f)", q=64),
                        hs[s_ * 64:(s_ + 1) * 64, :])
                    table = dr.tile([THROWS, F], dt.bfloat16,
                                    tag=f"table{li}_{s_}", addr_space="Shared")
                    nc.gpsimd.collective_compute(
                        "AllGather", ALU.bypass, replica_groups=rg,
                        ins=[bounce[:, :].opt()], outs=[table[:, :].opt()])
                    tables.append(table)

                if li == 1:
                    outsb = cp.tile([128, SPAD], dt.float32, tag="xn")

                # ---- message passing over windows ----
                ci = 0
                for wi, (w0, w1) in enumerate(p.windows):
                    Wc = w1 - w0
                    W = Wc * 128
                    ws = slice(w0 * 128, w1 * 128)
                    pb0, pb1 = p.wpairrange[wi]
                    d0 = int(p.dcoloff[pb0])
                    d1 = int(p.dcoloff[pb1])
                    dwin = dwp.tile([128, d1 - d0], dt.bfloat16, tag="dwin")
                    nc.sync.dma_start(dwin[:], dall_d.ap()[:, d0:d1])
                    msgs = {}
                    for s_ in (0, 1):
                        cl = p.calls[ci]; ci += 1
                        assert cl["s"] == s_ and cl["w0"] == w0
                        m = mp.tile([128, cl["G"], F], dt.bfloat16,
                                    tag=f"msg{s_}")
                        nc.gpsimd.dma_gather(
                            m[:, :, :], tables[s_][:, :],
                            idx_sb[s_][:, cl["off16"]:cl["off16"] + cl["L"] // 16],
                            num_idxs=cl["L"], num_idxs_reg=cl["L"],
                            elem_size=F, single_packet=False,
                            queue_num=(wi * 2 + s_) % 4)
                        msgs[s_] = m
                    tw = tp.tile([128, W], dt.float32, tag="tw")
                    for c in range(w0, w1):
                        cs = slice(c * 128, (c + 1) * 128)
                        acc = pacc.tile([128, F], dt.float32, tag="acc")
                        nc.tensor.matmul(acc[:], diag[:, cs], hs[:, cs],
                                         start=True, stop=False)
                        nc.tensor.matmul(acc[:], onesr[0:1, :], br[0:1, :],
                                         start=False, stop=False)
                        cps = p.chunk_pairs[c]
                        nc.tensor.matmul(acc[:], statT[:, cs], awr[:],
                                         start=False, stop=not cps)
                        for j, (s_, g, pi) in enumerate(cps):
                            lo = int(p.pair_lo[pi])
                            span = int(p.pair_span[pi])
                            loc = int(p.dcoloff[pi]) - d0
                            nc.tensor.matmul(
                                acc[lo:lo + span, :],
                                dwin[:, loc:loc + span],
                                msgs[s_][:, g, :],
                                start=False, stop=(j == len(cps) - 1))
                        nc.scalar.activation(
                            tw[:, (c - w0) * 128:(c - w0 + 1) * 128],
                            acc[:], AF.Tanh)
                    # ---- window epilogue ----
                    r01 = tp.tile([128, W], dt.float32, tag="r01")
                    nc.scalar.activation(r01[:], tw[:], AF.Copy, scale=EPS)
                    rx = tp.tile([128, W], dt.float32, tag="rx")
                    nc.vector.tensor_add(rx[:], r01[:], resid[:, ws])
                    if li == 0:
                        rr = tp.tile([128, W], dt.float32, tag="r01")
                        nc.scalar.activation(rr[:], rx[:], AF.Copy, scale=0.01)
                        x1 = tp.tile([128, W], dt.float32, tag="x1")
                        nc.vector.tensor_max(x1[:], rx[:], rr[:])
                        nc.vector.tensor_add(y2n[:, ws], x1[:], xn[:, ws])
                        for c in range(w0, w1):
                            cs = slice(c * 128, (c + 1) * 128)
                            pt = ph.tile([128, 128], dt.float32, tag="hp")
                            nc.tensor.transpose(pt[:], y2n[:, cs], ident[:])
                            nc.scalar.activation(y2T[:, cs], pt[:], AF.Copy)
                    else:
                        ex = tp.tile([128, W], dt.float32, tag="x1")
                        nc.scalar.activation(ex[:], rx[:], AF.Exp)
                        sm = scp.tile([128, Wc], dt.float32, tag="sm")
                        nc.vector.reduce_sum(
                            sm[:], ex[:].rearrange("p (c f) -> p c f", f=128),
                            axis=AX.X)
                        rc = scp.tile([128, Wc], dt.float32, tag="rc")
                        nc.vector.reciprocal(rc[:], sm[:])
                        for c in range(w0, w1):
                            cs = slice(c * 128, (c + 1) * 128)
                            j = c - w0
                            nc.scalar.activation(
                                outsb[:, cs], ex[:, j * 128:(j + 1) * 128],
                                AF.Copy, scale=rc[:, j:j + 1])

            nc.sync.dma_start(out_d.ap(), outsb[:])

    nc.compile()
    return nc


# ---------------------------------------------------------------------------
# Entry points
# ---------------------------------------------------------------------------


def _install_ntff_hook():
    """Register the axon NTFF profiling hook if the image lacks
    antenv.axon_hooks (lets run_bass_kernel_spmd(trace=True) return
    exec_time_ns + a perfetto trace)."""
    import types

    try:
        from antenv.axon_hooks import get_axon_ntff_profile_hook  # noqa: F401
        return
    except ImportError:
        pass
    try:
        import antenv

        store = {"h": None}
        mod = types.ModuleType("antenv.axon_hooks")
        mod.set_axon_ntff_profile_hook = lambda h: store.__setitem__("h", h)
        mod.get_axon_ntff_profile_hook = lambda: store["h"]
        sys.modules["antenv.axon_hooks"] = mod
        antenv.axon_hooks = mod
        if "/root/.axon_site" not in sys.path:
            sys.path.insert(0, "/root/.axon_site")
        from trn_agent_boot import trn_boot as tb

        hook = tb._ntff_profile_via_ctypes("/opt/axon/libaxon_pjrt.so")
        mod.set_axon_ntff_profile_hook(hook)
    except Exception as e:  # profiling is best-effort
        print(f"ntff hook install failed: {e}")


def _run(inputs, n_cores=8, trace=None):
    from concourse import bass_utils

    x = np.asarray(inputs["x"], np.float32)
    W1 = np.asarray(inputs["W1"], np.float32)
    b1 = np.asarray(inputs["b1"], np.float32)
    lin1 = np.asarray(inputs["lin1"], np.float32)
    W2 = np.asarray(inputs["W2"], np.float32)
    b2 = np.asarray(inputs["b2"], np.float32)
    lin2 = np.asarray(inputs["lin2"], np.float32)
    ei = np.asarray(inputs["edge_index"])

    N = x.shape[0]
    p = _host_prep(ei, N, n_cores)
    in_maps = _per_core_inputs(p, x, W1, b1, lin1, W2, b2, lin2)
    nc = _build(p)

    if trace is None:
        trace = os.environ.get("KERNEL_TRACE", "0") == "1"
    if trace:
        _install_ntff_hook()
    res = bass_utils.run_bass_kernel_spmd(
        nc, in_maps, core_ids=list(range(n_cores)), trace=trace)
    LAST_EXEC_NS["ns"] = res.exec_time_ns

    outs = []
    for i in range(n_cores):
        o = res.results[i]["out"]  # [128, SPAD] partition-major
        o = o.reshape(128, p.NCH, 128).transpose(1, 0, 2).reshape(p.SPAD, 128)
        outs.append(o[:p.S])
    return np.concatenate(outs, axis=0).astype(np.float32)


def kernel(**inputs):
    return _run(inputs)


# revision 19
# speedup vs baseline: 1.0094x; 1.0094x over previous
"""Distributed Bass kernel for 2-layer AntiSymmetricConv GNN on 8 TRN2 NeuronCores.

Strategy (graph/data parallel, dst-sharded):
  - Nodes sharded across 8 cores (6250 each, padded to 6272 = 49 chunks of 128).
  - Per layer: each core computes hs = (y @ lin^T) * dinv for its shard (PE),
    AllGathers the bf16 table (Shared addr space) to every core's DRAM.
  - Edges are assigned to the core owning dst. Messages hs[src] are fetched with
    SWDGE dma_gather (arbitrary-index row gather), round-robined over 4 SWDGE
    queues for drain parallelism.
  - Scatter-add runs on the TensorEngine: per 128-edge group a span-packed
    one-hot matrix D (host-built, streamed from DRAM per window, entries
    dinv[dst]) is the stationary operand; the gathered msg tile is the moving
    operand; PSUM accumulates [dst-node x feat] chunks.  Self-loop term enters
    as a host-built diag(dinv) stationary; bias as a K=1 outer-product matmul;
    the antisymmetric term x @ aw^T as an x^T-chunk stationary matmul.
  - Epilogue (tanh/residual/leaky-relu, final softmax) on ACT + DVE at window
    granularity.
All index preprocessing (degrees, edge->core/chunk assignment, gather indices,
one-hot construction) happens on the host from edge_index only.
"""

import os
import sys

sys.path.insert(0, "/opt/trn_rl_repo")

import numpy as np
import ml_dtypes

BF16 = ml_dtypes.bfloat16
EPS = 0.1
GAMMA = 0.1

LAST_EXEC_NS = {"ns": None}


def _ceil(a, b):
    return -(-a // b)


# ---------------------------------------------------------------------------
# Host-side preprocessing (index-only work + weight re-layout)
# ---------------------------------------------------------------------------


class Prep:
    pass


def _host_prep(edge_index, N, n_cores, wchunks=4):
    """Build the shared (core-independent) schedule and per-core index data.

    Subsets are src partition-halves (q<64 / q>=64): each subset has its own
    half-table (AllGather of 64 hs partitions), so the two AGs per layer can
    overlap with the other subset's gathers.  Gather padding happens at
    (window, subset) granularity; 128-edge groups may straddle chunk
    boundaries, yielding one D pair per (group, chunk) combination.
    """
    p = Prep()
    S = N // n_cores
    assert S * n_cores == N
    SPAD = _ceil(S, 128) * 128
    NCH = SPAD // 128
    HROWS = 64 * NCH           # rows per rank per half-table
    THROWS = n_cores * HROWS   # half-table rows
    assert THROWS < 32768

    src = np.asarray(edge_index[0], dtype=np.int64)
    dst = np.asarray(edge_index[1], dtype=np.int64)
    E = src.shape[0]

    deg = np.bincount(dst, minlength=N).astype(np.float64) + 1.0
    dinv = (1.0 / np.sqrt(deg)).astype(np.float32)

    core = dst // S
    dloc = dst - core * S
    chunk = dloc // 128
    sl = src % S
    sc = src // S
    sq = sl % 128
    sub = sq // 64
    trow = sc * HROWS + (sq - 64 * sub) * NCH + (sl // 128)

    windows = [(a, min(a + wchunks, NCH)) for a in range(0, NCH, wchunks)]
    wof = np.zeros(NCH, np.int64)   # window index per chunk
    for wi, (w0, w1) in enumerate(windows):
        wof[w0:w1] = wi
    ewin = wof[chunk]

    NW = len(windows)

    # slot of each edge's source row within its (core, window, sub) gather
    # stream: rows deduped per stream (multiple edges may share one gathered
    # row), ordered by (chunk of first use, trow)
    flat = (core * NW + ewin) * 2 + sub
    order0 = np.lexsort((dloc, trow, chunk, flat))  # unique (chunk, trow) rows
    sf = flat[order0]
    st = trow[order0]
    sd = dloc[order0]
    sch = chunk[order0]
    new_row = np.concatenate(
        [[True], (sf[1:] != sf[:-1]) | (st[1:] != st[:-1])
         | (sch[1:] != sch[:-1])])
    rid_sorted = np.cumsum(new_row) - 1       # row id, (flat, trow) order
    nrows = int(rid_sorted[-1]) + 1
    rows_flat = sf[new_row]
    rows_trow = st[new_row]
    rows_mindloc = sd[new_row]                # min dloc among the row's edges
    # order rows within each stream by (min dloc, trow) to keep D spans small
    ro = np.lexsort((rows_trow, rows_mindloc, rows_flat))
    rank = np.empty(nrows, np.int64)
    rank[ro] = np.arange(nrows)
    seg_first_rank = np.zeros(n_cores * NW * 2, np.int64)
    rf_sorted = rows_flat[ro]
    first = np.concatenate([[True], rf_sorted[1:] != rf_sorted[:-1]])
    seg_first_rank[rf_sorted[first]] = np.flatnonzero(first)
    pos_row = rank - seg_first_rank[rows_flat]  # slot within stream
    e_rid = np.empty(E, np.int64)
    e_rid[order0] = rid_sorted
    pos = pos_row[e_rid]
    segcnt = np.zeros(n_cores * NW * 2, np.int64)
    np.maximum.at(segcnt, flat, pos + 1)
    segcnt = segcnt.reshape(n_cores, NW, 2)

    Lws = 128 * np.maximum(_ceil(segcnt.max(axis=0), 128), 1)  # [NW, 2]

    e_group = pos // 128
    e_pp = pos % 128
    e_col = dloc - chunk * 128

    # pair = (window, sub, group, chunk); build union over cores
    gmax = int(Lws.max()) // 128
    pair_of = {}
    pair_list = []  # (wi, s, g, c)
    seen = np.zeros((NW, 2, gmax, wchunks), bool)
    np.logical_or.at(seen, (ewin, sub, e_group, chunk - np.array([w[0] for w in windows])[ewin]), True)
    for wi in range(NW):
        w0, w1 = windows[wi]
        for c in range(w0, w1):
            for s in (0, 1):
                for g in range(Lws[wi, s] // 128):
                    if seen[wi, s, g, c - w0]:
                        pair_of[(wi, s, g, c)] = len(pair_list)
                        pair_list.append((wi, s, g, c))
    npair = len(pair_list)

    # vectorized pair id lookup
    pid_lut = np.full((NW, 2, gmax, wchunks), -1, np.int64)
    for idx_, (wi, s, g, c) in enumerate(pair_list):
        pid_lut[wi, s, g, c - windows[wi][0]] = idx_
    e_pair = pid_lut[ewin, sub, e_group,
                     chunk - np.array([w[0] for w in windows])[ewin]]
    assert (e_pair >= 0).all()

    pair_lo = np.full(npair, 128, np.int64)
    pair_hi = np.full(npair, -1, np.int64)
    np.minimum.at(pair_lo, e_pair, e_col)
    np.maximum.at(pair_hi, e_pair, e_col)
    empty = pair_hi < 0
    pair_lo[empty] = 0
    pair_hi[empty] = 0
    # PE matmul output row-group rules: base 0 (<=128 rows), base 32
    # (<=32 rows, i.e. hi<64), base 64 (<=64 rows)
    base = np.zeros(npair, np.int64)
    base[(pair_lo >= 32) & (pair_hi < 64)] = 32
    base[pair_lo >= 64] = 64
    pair_lo = base
    pair_span = pair_hi - pair_lo + 1
    dcoloff = np.concatenate([[0], np.cumsum(pair_span)])
    DCOLS = int(dcoloff[-1])

    # per-window D column range and per-chunk pair emission lists
    wpairrange = []
    chunk_pairs = [[] for _ in range(NCH)]  # (s, g, pi)
    pi0 = 0
    for wi in range(NW):
        w0, w1 = windows[wi]
        lo = npair
        hi = 0
        for c in range(w0, w1):
            for s in (0, 1):
                for g in range(Lws[wi, s] // 128):
                    key = (wi, s, g, c)
                    if key in pair_of:
                        pi = pair_of[key]
                        chunk_pairs[c].append((s, g, pi))
                        lo = min(lo, pi)
                        hi = max(hi, pi + 1)
        wpairrange.append((lo, hi))
    # pairs are emitted in (wi, ...) blocks; verify contiguity per window
    for wi, (lo, hi) in enumerate(wpairrange):
        for idx_ in range(lo, hi):
            assert pair_list[idx_][0] == wi

    # gather calls: one per (window, subset)
    calls = []
    off16 = [0, 0]
    for wi in range(NW):
        w0, w1 = windows[wi]
        for s in (0, 1):
            L = int(Lws[wi, s])
            calls.append(dict(w0=w0, w1=w1, s=s, off16=off16[s], L=L,
                              G=L // 128))
            off16[s] += L // 16
    idx_cols16 = [off16[0], off16[1]]
    segoff = np.zeros((NW, 2), np.int64)
    ci = 0
    for wi in range(NW):
        for s in (0, 1):
            segoff[wi, s] = calls[ci]["off16"] * 16
            ci += 1

    idx_arrs = []
    dall_arrs = []
    for i in range(n_cores):
        m = core == i
        idxs = [np.zeros(idx_cols16[0] * 16, np.int16),
                np.zeros(idx_cols16[1] * 16, np.int16)]
        gpos = segoff[ewin[m], sub[m]] + pos[m]
        tr = trow[m].astype(np.int16)
        for s in (0, 1):
            ms = sub[m] == s
            idxs[s][gpos[ms]] = tr[ms]
        dall = np.zeros((128, DCOLS), np.float32)
        np.add.at(dall, (e_pp[m], dcoloff[e_pair[m]]
                         + (e_col[m] - pair_lo[e_pair[m]])), dinv[dst[m]])
        wrapped = []
        for s in (0, 1):
            a = idxs[s].reshape(-1, 16).T
            wrapped.append(np.tile(a, (8, 1)).astype(np.int16))
        idx_arrs.append(wrapped)
        dall_arrs.append(dall.astype(BF16))

    p.S, p.SPAD, p.NCH = S, SPAD, NCH
    p.HROWS, p.THROWS = HROWS, THROWS
    p.E, p.N, p.n_cores = E, N, n_cores
    p.dinv = dinv
    p.windows, p.calls, p.npair = windows, calls, npair
    p.idx_cols16 = idx_cols16
    p.wpairrange, p.pair_lo, p.pair_span, p.dcoloff, p.DCOLS = (
        wpairrange, pair_lo, pair_span, dcoloff, DCOLS)
    p.chunk_pairs = chunk_pairs
    p.idx_arrs, p.dall_arrs = idx_arrs, dall_arrs
    return p


def _per_core_inputs(p, x, W1, b1, lin1, W2, b2, lin2):
    """Build in_maps (list of dicts) for run_bass_kernel_spmd."""
    F = x.shape[1]
    S, SPAD, NCH = p.S, p.SPAD, p.NCH
    aw1 = W1 - W1.T - GAMMA * np.eye(F, dtype=np.float32)
    aw2 = W2 - W2.T - GAMMA * np.eye(F, dtype=np.float32)
    ident = np.eye(128, dtype=np.float32)
    in_maps = []
    for i in range(p.n_cores):
        xi = np.zeros((SPAD, F), np.float32)
        xi[:S] = x[i * S:(i + 1) * S]
        dinv_i = np.ones(SPAD, np.float32)
        dinv_i[:S] = p.dinv[i * S:(i + 1) * S]
        x_pm = xi.reshape(NCH, 128, F).transpose(1, 0, 2).reshape(128, NCH * F)
        dinv_col = dinv_i.reshape(NCH, 128).T.copy()  # [128, NCH]
        diag = np.zeros((128, NCH, 128), np.float32)
        rng = np.arange(128)
        diag[rng, :, rng] = dinv_col
        in_maps.append({
            "xT": np.ascontiguousarray(xi.T).astype(BF16),
            "xn": x_pm.astype(np.float32),
            "dinvc": dinv_col.astype(np.float32),
            "diag": diag.reshape(128, NCH * 128).astype(BF16),
            "lin1r": lin1.T.astype(BF16), "aw1r": aw1.T.astype(BF16),
            "lin2r": lin2.T.astype(BF16), "aw2r": aw2.T.astype(BF16),
            "b1r": b1.reshape(1, F).astype(BF16),
            "b2r": b2.reshape(1, F).astype(BF16),
            "onesr": np.ones((1, 128), BF16),
            "ident": ident,
            "idxlo": p.idx_arrs[i][0], "idxhi": p.idx_arrs[i][1],
            "dall": p.dall_arrs[i],
        })
    return in_maps


# ---------------------------------------------------------------------------
# Bass graph
# ---------------------------------------------------------------------------


def _build(p):
    import concourse.bass as bass
    import concourse.bacc as bacc
    import concourse.mybir as mybir
    import concourse.tile as tile

    dt = mybir.dt
    F = 128
    SPAD, NCH = p.SPAD, p.NCH
    HROWS, THROWS = p.HROWS, p.THROWS

    nc = bacc.Bacc("TRN2", target_bir_lowering=False, debug=False,
                   num_devices=p.n_cores, num_swdge_queues=4)

    def din(name, shape, d):
        return nc.dram_tensor(name, shape, d, kind="ExternalInput")

    xT_d = din("xT", [F, SPAD], dt.bfloat16)
    xn_d = din("xn", [128, SPAD], dt.float32)
    dinvc_d = din("dinvc", [128, NCH], dt.float32)
    diag_d = din("diag", [128, NCH * 128], dt.bfloat16)
    lin1r_d = din("lin1r", [F, F], dt.bfloat16)
    aw1r_d = din("aw1r", [F, F], dt.bfloat16)
    lin2r_d = din("lin2r", [F, F], dt.bfloat16)
    aw2r_d = din("aw2r", [F, F], dt.bfloat16)
    b1r_d = din("b1r", [1, F], dt.bfloat16)
    b2r_d = din("b2r", [1, F], dt.bfloat16)
    ones_d = din("onesr", [1, 128], dt.bfloat16)
    ident_d = din("ident", [128, 128], dt.float32)
    idxlo_d = din("idxlo", [128, p.idx_cols16[0]], dt.int16)
    idxhi_d = din("idxhi", [128, p.idx_cols16[1]], dt.int16)
    dall_d = din("dall", [128, p.DCOLS], dt.bfloat16)
    out_d = nc.dram_tensor("out", [128, SPAD], dt.float32, kind="ExternalOutput")

    AF = mybir.ActivationFunctionType
    ALU = mybir.AluOpType
    AX = mybir.AxisListType

    with tile.TileContext(nc) as tc:
        with tc.tile_pool(name="const", bufs=1) as cp, \
             tc.tile_pool(name="msg", bufs=4) as mp, \
             tc.tile_pool(name="dwin", bufs=2) as dwp, \
             tc.tile_pool(name="tmp", bufs=2) as tp, \
             tc.tile_pool(name="sc", bufs=3) as scp, \
             tc.tile_pool(name="acc", bufs=4, space="PSUM") as pacc, \
             tc.tile_pool(name="psh", bufs=2, space="PSUM") as ph, \
             tc.tile_pool(name="dram", bufs=1, space="DRAM") as dr:

            def load(d, shape, dtp, tag):
                t = cp.tile(shape, dtp, tag=tag)
                nc.sync.dma_start(t[:], d.ap())
                return t

            xT = load(xT_d, [F, SPAD], dt.bfloat16, "xT")
            xn = load(xn_d, [128, SPAD], dt.float32, "xn")
            dinvc = load(dinvc_d, [128, NCH], dt.float32, "dinvc")
            diag = load(diag_d, [128, NCH * 128], dt.bfloat16, "diag")
            lin1r = load(lin1r_d, [F, F], dt.bfloat16, "lin1r")
            aw1r = load(aw1r_d, [F, F], dt.bfloat16, "aw1r")
            lin2r = load(lin2r_d, [F, F], dt.bfloat16, "lin2r")
            aw2r = load(aw2r_d, [F, F], dt.bfloat16, "aw2r")
            b1r = load(b1r_d, [1, F], dt.bfloat16, "b1r")
            b2r = load(b2r_d, [1, F], dt.bfloat16, "b2r")
            onesr = load(ones_d, [1, 128], dt.bfloat16, "onesr")
            ident = load(ident_d, [128, 128], dt.float32, "ident")
            idx_sb = [load(idxlo_d, [128, p.idx_cols16[0]], dt.int16, "idxlo"),
                      load(idxhi_d, [128, p.idx_cols16[1]], dt.int16, "idxhi")]

            y2n = cp.tile([128, SPAD], dt.float32, tag="y2n")
            y2T = cp.tile([F, SPAD], dt.bfloat16, tag="y2T")
            outsb = None

            rg = [list(range(p.n_cores))]

            for li in range(2):
                statT = xT if li == 0 else y2T
                resid = xn if li == 0 else y2n
                linr = lin1r if li == 0 else lin2r
                awr = aw1r if li == 0 else aw2r
                br = b1r if li == 0 else b2r

                # ---- hs = (y @ lin^T) * dinv  (bf16, node-orient) ----
                hs = cp.tile([128, SPAD], dt.bfloat16, tag="hs")
                for c in range(NCH):
                    cs = slice(c * 128, (c + 1) * 128)
                    hp = ph.tile([128, F], dt.float32, tag="hp")
                    nc.tensor.matmul(hp[:], statT[:, cs], linr[:],
                                     start=True, stop=True)
                    nc.scalar.activation(hs[:, cs], hp[:], AF.Copy,
                                         scale=dinvc[:, c:c + 1])

                # ---- AllGather bf16 half-tables ----
                tables = []
                for s_ in (0, 1):
                    bounce = dr.tile([HROWS, F], dt.bfloat16,
                                     tag=f"bounce{li}_{s_}")
                    nc.sync.dma_start(
                        bounce[:, :].rearrange("(q c) f -> q (c f)", q=64),
                        hs[s_ * 64:(s_ + 1) * 64, :])
                    table = dr.tile([THROWS, F], dt.bfloat16,
                                    tag=f"table{li}_{s_}", addr_space="Shared")
                    nc.gpsimd.collective_compute(
                        "AllGather", ALU.bypass, replica_groups=rg,
                        ins=[bounce[:, :].opt()], outs=[table[:, :].opt()])
                    tables.append(table)

                if li == 1:
                    outsb = cp.tile([128, SPAD], dt.float32, tag="xn")

                # ---- message passing over windows ----
                ci = 0
                for wi, (w0, w1) in enumerate(p.windows):
                    Wc = w1 - w0
                    W = Wc * 128
                    ws = slice(w0 * 128, w1 * 128)
                    pb0, pb1 = p.wpairrange[wi]
                    d0 = int(p.dcoloff[pb0])
                    d1 = int(p.dcoloff[pb1])
                    dwin = dwp.tile([128, d1 - d0], dt.bfloat16, tag="dwin")
                    nc.sync.dma_start(dwin[:], dall_d.ap()[:, d0:d1])
                    msgs = {}
                    for s_ in (0, 1):
                        cl = p.calls[ci]; ci += 1
                        assert cl["s"] == s_ and cl["w0"] == w0
                        m = mp.tile([128, cl["G"], F], dt.bfloat16,
                                    tag=f"msg{s_}")
                        nc.gpsimd.dma_gather(
                            m[:, :, :], tables[s_][:, :],
                            idx_sb[s_][:, cl["off16"]:cl["off16"] + cl["L"] // 16],
                            num_idxs=cl["L"], num_idxs_reg=cl["L"],
                            elem_size=F, single_packet=False,
                            queue_num=(wi * 2 + s_) % 4)
                        msgs[s_] = m
                    tw = tp.tile([128, W], dt.float32, tag="tw")
                    for c in range(w0, w1):
                        cs = slice(c * 128, (c + 1) * 128)
                        acc = pacc.tile([128, F], dt.float32, tag="acc")
                        nc.tensor.matmul(acc[:], diag[:, cs], hs[:, cs],
                                         start=True, stop=False)
                        nc.tensor.matmul(acc[:], onesr[0:1, :], br[0:1, :],
                                         start=False, stop=False)
                        cps = p.chunk_pairs[c]
                        nc.tensor.matmul(acc[:], statT[:, cs], awr[:],
                                         start=False, stop=not cps)
                        for j, (s_, g, pi) in enumerate(cps):
                            lo = int(p.pair_lo[pi])
                            span = int(p.pair_span[pi])
                            loc = int(p.dcoloff[pi]) - d0
                            nc.tensor.matmul(
                                acc[lo:lo + span, :],
                                dwin[:, loc:loc + span],
                                msgs[s_][:, g, :],
                                start=False, stop=(j == len(cps) - 1))
                        nc.scalar.activation(
                            tw[:, (c - w0) * 128:(c - w0 + 1) * 128],
                            acc[:], AF.Tanh)
                    # ---- window epilogue ----
                    r01 = tp.tile([128, W], dt.float32, tag="r01")
                    nc.scalar.activation(r01[:], tw[:], AF.Copy, scale=EPS)
                    rx = tp.tile([128, W], dt.float32, tag="rx")
                    nc.vector.tensor_add(rx[:], r01[:], resid[:, ws])
                    if li == 0:
                        rr = tp.tile([128, W], dt.float32, tag="r01")
                        nc.scalar.activation(rr[:], rx[:], AF.Copy, scale=0.01)
                        x1 = tp.tile([128, W], dt.float32, tag="x1")
                        nc.vector.tensor_max(x1[:], rx[:], rr[:])
                        nc.vector.tensor_add(y2n[:, ws], x1[:], xn[:, ws])
                        for c in range(w0, w1):
                            cs = slice(c * 128, (c + 1) * 128)
                            pt = ph.tile([128, 128], dt.float32, tag="hp")
                            nc.tensor.transpose(pt[:], y2n[:, cs], ident[:])
                            nc.scalar.activation(y2T[:, cs], pt[:], AF.Copy)
                    else:
                        ex = tp.tile([128, W], dt.float32, tag="x1")
                        nc.scalar.activation(ex[:], rx[:], AF.Exp)
                        sm = scp.tile([128, Wc], dt.float32, tag="sm")
                        nc.vector.reduce_sum(
                            sm[:], ex[:].rearrange("p (c f) -> p c f", f=128),
                            axis=AX.X)
                        rc = scp.tile([128, Wc], dt.float32, tag="rc")
                        nc.vector.reciprocal(rc[:], sm[:])
                        for c in range(w0, w1):
                            cs = slice(c * 128, (c + 1) * 128)
                            j = c - w0
                            nc.scalar.activation(
                                outsb[:, cs], ex[:, j * 128:(j + 1) * 128],
                                AF.Copy, scale=rc[:, j:j + 1])

            nc.sync.dma_start(out_d.ap(), outsb[:])

    nc.compile()
    return nc


# ---------------------------------------------------------------------------
# Entry points
# ---------------------------------------------------------------------------


def _install_ntff_hook():
    """Register the axon NTFF profiling hook if the image lacks
    antenv.axon_hooks (lets run_bass_kernel_spmd(trace=True) return
    exec_time_ns + a perfetto trace)."""
    import types

    try:
        from antenv.axon_hooks import get_axon_ntff_profile_hook  # noqa: F401
        return
    except ImportError:
        pass
    try:
        import antenv

        store = {"h": None}
        mod = types.ModuleType("antenv.axon_hooks")
        mod.set_axon_ntff_profile_hook = lambda h: store.__setitem__("h", h)
        mod.get_axon_ntff_profile_hook = lambda: store["h"]
        sys.modules["antenv.axon_hooks"] = mod
        antenv.axon_hooks = mod
        if "/root/.axon_site" not in sys.path:
            sys.path.insert(0, "/root/.axon_site")
        from trn_agent_boot import trn_boot as tb

        hook = tb._ntff_profile_via_ctypes("/opt/axon/libaxon_pjrt.so")
        mod.set_axon_ntff_profile_hook(hook)
    except Exception as e:  # profiling is best-effort
        print(f"ntff hook install failed: {e}")


def _run(inputs, n_cores=8, trace=None):
    from concourse import bass_utils

    x = np.asarray(inputs["x"], np.float32)
    W1 = np.asarray(inputs["W1"], np.float32)
    b1 = np.asarray(inputs["b1"], np.float32)
    lin1 = np.asarray(inputs["lin1"], np.float32)
    W2 = np.asarray(inputs["W2"], np.float32)
    b2 = np.asarray(inputs["b2"], np.float32)
    lin2 = np.asarray(inputs["lin2"], np.float32)
    ei = np.asarray(inputs["edge_index"])

    N = x.shape[0]
    p = _host_prep(ei, N, n_cores)
    in_maps = _per_core_inputs(p, x, W1, b1, lin1, W2, b2, lin2)
    nc = _build(p)

    if trace is None:
        trace = os.environ.get("KERNEL_TRACE", "0") == "1"
    if trace:
        _install_ntff_hook()
    res = bass_utils.run_bass_kernel_spmd(
        nc, in_maps, core_ids=list(range(n_cores)), trace=trace)
    LAST_EXEC_NS["ns"] = res.exec_time_ns

    outs = []
    for i in range(n_cores):
        o = res.results[i]["out"]  # [128, SPAD] partition-major
        o = o.reshape(128, p.NCH, 128).transpose(1, 0, 2).reshape(p.SPAD, 128)
        outs.append(o[:p.S])
    return np.concatenate(outs, axis=0).astype(np.float32)


def kernel(**inputs):
    return _run(inputs)


# revision 20
# speedup vs baseline: 1.1229x; 1.1125x over previous
"""Distributed Bass kernel for 2-layer AntiSymmetricConv GNN on 8 TRN2 NeuronCores.

Strategy (graph/data parallel, dst-sharded):
  - Nodes sharded across 8 cores (6250 each, padded to 6272 = 49 chunks of 128).
  - Per layer: each core computes hs = (y @ lin^T) * dinv for its shard (PE),
    AllGathers the bf16 table (Shared addr space) to every core's DRAM.
  - Edges are assigned to the core owning dst. Messages hs[src] are fetched with
    SWDGE dma_gather (arbitrary-index row gather), round-robined over 4 SWDGE
    queues for drain parallelism.
  - Scatter-add runs on the TensorEngine: per 128-edge group a span-packed
    one-hot matrix D (host-built, streamed from DRAM per window, entries
    dinv[dst]) is the stationary operand; the gathered msg tile is the moving
    operand; PSUM accumulates [dst-node x feat] chunks.  Self-loop term enters
    as a host-built diag(dinv) stationary; bias as a K=1 outer-product matmul;
    the antisymmetric term x @ aw^T as an x^T-chunk stationary matmul.
  - Epilogue (tanh/residual/leaky-relu, final softmax) on ACT + DVE at window
    granularity.
All index preprocessing (degrees, edge->core/chunk assignment, gather indices,
one-hot construction) happens on the host from edge_index only.
"""

import os
import sys

sys.path.insert(0, "/opt/trn_rl_repo")

import numpy as np
import ml_dtypes

BF16 = ml_dtypes.bfloat16
EPS = 0.1
GAMMA = 0.1

LAST_EXEC_NS = {"ns": None}


def _ceil(a, b):
    return -(-a // b)


# ---------------------------------------------------------------------------
# Host-side preprocessing (index-only work + weight re-layout)
# ---------------------------------------------------------------------------


class Prep:
    pass


def _host_prep(edge_index, N, n_cores, wchunks=4):
    """Build the shared (core-independent) schedule and per-core index data.

    Subsets are src partition-halves (q<64 / q>=64): each subset has its own
    half-table (AllGather of 64 hs partitions), so the two AGs per layer can
    overlap with the other subset's gathers.  Gather padding happens at
    (window, subset) granularity; 128-edge groups may straddle chunk
    boundaries, yielding one D pair per (group, chunk) combination.
    """
    p = Prep()
    S = N // n_cores
    assert S * n_cores == N
    SPAD = _ceil(S, 128) * 128
    NCH = SPAD // 128
    HROWS = 64 * NCH           # rows per rank per half-table
    THROWS = n_cores * HROWS   # half-table rows
    assert THROWS < 32768

    src = np.asarray(edge_index[0], dtype=np.int64)
    dst = np.asarray(edge_index[1], dtype=np.int64)
    E = src.shape[0]

    deg = np.bincount(dst, minlength=N).astype(np.float64) + 1.0
    dinv = (1.0 / np.sqrt(deg)).astype(np.float32)

    core = dst // S
    dloc = dst - core * S
    chunk = dloc // 128
    sl = src % S
    sc = src // S
    sq = sl % 128
    sub = sq // 64
    trow = sc * HROWS + (sq - 64 * sub) * NCH + (sl // 128)

    windows = [(a, min(a + wchunks, NCH)) for a in range(0, NCH, wchunks)]
    wof = np.zeros(NCH, np.int64)   # window index per chunk
    for wi, (w0, w1) in enumerate(windows):
        wof[w0:w1] = wi
    ewin = wof[chunk]

    NW = len(windows)

    # slot of each edge's source row within its (core, window, sub) gather
    # stream: rows deduped per stream (multiple edges may share one gathered
    # row), ordered by (chunk of first use, trow)
    flat = (core * NW + ewin) * 2 + sub
    order0 = np.lexsort((dloc, trow, chunk, flat))  # unique (chunk, trow) rows
    sf = flat[order0]
    st = trow[order0]
    sd = dloc[order0]
    sch = chunk[order0]
    new_row = np.ones(E, bool)  # no dedupe: one gathered row per edge
    rid_sorted = np.cumsum(new_row) - 1       # row id, (flat, trow) order
    nrows = int(rid_sorted[-1]) + 1
    rows_flat = sf[new_row]
    rows_trow = st[new_row]
    rows_mindloc = sd[new_row]                # min dloc among the row's edges
    # order rows within each stream by (min dloc, trow) to keep D spans small
    ro = np.lexsort((rows_trow, rows_mindloc, rows_flat))
    rank = np.empty(nrows, np.int64)
    rank[ro] = np.arange(nrows)
    seg_first_rank = np.zeros(n_cores * NW * 2, np.int64)
    rf_sorted = rows_flat[ro]
    first = np.concatenate([[True], rf_sorted[1:] != rf_sorted[:-1]])
    seg_first_rank[rf_sorted[first]] = np.flatnonzero(first)
    pos_row = rank - seg_first_rank[rows_flat]  # slot within stream
    e_rid = np.empty(E, np.int64)
    e_rid[order0] = rid_sorted
    pos = pos_row[e_rid]
    segcnt = np.zeros(n_cores * NW * 2, np.int64)
    np.maximum.at(segcnt, flat, pos + 1)
    segcnt = segcnt.reshape(n_cores, NW, 2)

    Lws = 128 * np.maximum(_ceil(segcnt.max(axis=0), 128), 1)  # [NW, 2]

    e_group = pos // 128
    e_pp = pos % 128
    e_col = dloc - chunk * 128

    # pair = (window, sub, group, chunk); build union over cores
    gmax = int(Lws.max()) // 128
    pair_of = {}
    pair_list = []  # (wi, s, g, c)
    seen = np.zeros((NW, 2, gmax, wchunks), bool)
    np.logical_or.at(seen, (ewin, sub, e_group, chunk - np.array([w[0] for w in windows])[ewin]), True)
    for wi in range(NW):
        w0, w1 = windows[wi]
        for c in range(w0, w1):
            for s in (0, 1):
                for g in range(Lws[wi, s] // 128):
                    if seen[wi, s, g, c - w0]:
                        pair_of[(wi, s, g, c)] = len(pair_list)
                        pair_list.append((wi, s, g, c))
    npair = len(pair_list)

    # vectorized pair id lookup
    pid_lut = np.full((NW, 2, gmax, wchunks), -1, np.int64)
    for idx_, (wi, s, g, c) in enumerate(pair_list):
        pid_lut[wi, s, g, c - windows[wi][0]] = idx_
    e_pair = pid_lut[ewin, sub, e_group,
                     chunk - np.array([w[0] for w in windows])[ewin]]
    assert (e_pair >= 0).all()

    pair_lo = np.full(npair, 128, np.int64)
    pair_hi = np.full(npair, -1, np.int64)
    np.minimum.at(pair_lo, e_pair, e_col)
    np.maximum.at(pair_hi, e_pair, e_col)
    empty = pair_hi < 0
    pair_lo[empty] = 0
    pair_hi[empty] = 0
    # PE matmul output row-group rules: base 0 (<=128 rows), base 32
    # (<=32 rows, i.e. hi<64), base 64 (<=64 rows)
    base = np.zeros(npair, np.int64)
    base[(pair_lo >= 32) & (pair_hi < 64)] = 32
    base[pair_lo >= 64] = 64
    pair_lo = base
    pair_span = pair_hi - pair_lo + 1
    dcoloff = np.concatenate([[0], np.cumsum(pair_span)])
    DCOLS = int(dcoloff[-1])

    # per-window D column range and per-chunk pair emission lists
    wpairrange = []
    chunk_pairs = [[] for _ in range(NCH)]  # (s, g, pi)
    pi0 = 0
    for wi in range(NW):
        w0, w1 = windows[wi]
        lo = npair
        hi = 0
        for c in range(w0, w1):
            for s in (0, 1):
                for g in range(Lws[wi, s] // 128):
                    key = (wi, s, g, c)
                    if key in pair_of:
                        pi = pair_of[key]
                        chunk_pairs[c].append((s, g, pi))
                        lo = min(lo, pi)
                        hi = max(hi, pi + 1)
        wpairrange.append((lo, hi))
    # pairs are emitted in (wi, ...) blocks; verify contiguity per window
    for wi, (lo, hi) in enumerate(wpairrange):
        for idx_ in range(lo, hi):
            assert pair_list[idx_][0] == wi

    # gather calls: one per (window, subset)
    calls = []
    off16 = [0, 0]
    for wi in range(NW):
        w0, w1 = windows[wi]
        for s in (0, 1):
            L = int(Lws[wi, s])
            calls.append(dict(w0=w0, w1=w1, s=s, off16=off16[s], L=L,
                              G=L // 128))
            off16[s] += L // 16
    idx_cols16 = [off16[0], off16[1]]
    segoff = np.zeros((NW, 2), np.int64)
    ci = 0
    for wi in range(NW):
        for s in (0, 1):
            segoff[wi, s] = calls[ci]["off16"] * 16
            ci += 1

    idx_arrs = []
    dall_arrs = []
    for i in range(n_cores):
        m = core == i
        idxs = [np.zeros(idx_cols16[0] * 16, np.int16),
                np.zeros(idx_cols16[1] * 16, np.int16)]
        gpos = segoff[ewin[m], sub[m]] + pos[m]
        tr = trow[m].astype(np.int16)
        for s in (0, 1):
            ms = sub[m] == s
            idxs[s][gpos[ms]] = tr[ms]
        dall = np.zeros((128, DCOLS), np.float32)
        np.add.at(dall, (e_pp[m], dcoloff[e_pair[m]]
                         + (e_col[m] - pair_lo[e_pair[m]])), dinv[dst[m]])
        wrapped = []
        for s in (0, 1):
            a = idxs[s].reshape(-1, 16).T
            wrapped.append(np.tile(a, (8, 1)).astype(np.int16))
        idx_arrs.append(wrapped)
        dall_arrs.append(dall.astype(BF16))

    p.S, p.SPAD, p.NCH = S, SPAD, NCH
    p.HROWS, p.THROWS = HROWS, THROWS
    p.E, p.N, p.n_cores = E, N, n_cores
    p.dinv = dinv
    p.windows, p.calls, p.npair = windows, calls, npair
    p.idx_cols16 = idx_cols16
    p.wpairrange, p.pair_lo, p.pair_span, p.dcoloff, p.DCOLS = (
        wpairrange, pair_lo, pair_span, dcoloff, DCOLS)
    p.chunk_pairs = chunk_pairs
    p.idx_arrs, p.dall_arrs = idx_arrs, dall_arrs
    return p


def _per_core_inputs(p, x, W1, b1, lin1, W2, b2, lin2):
    """Build in_maps (list of dicts) for run_bass_kernel_spmd."""
    F = x.shape[1]
    S, SPAD, NCH = p.S, p.SPAD, p.NCH
    aw1 = W1 - W1.T - GAMMA * np.eye(F, dtype=np.float32)
    aw2 = W2 - W2.T - GAMMA * np.eye(F, dtype=np.float32)
    ident = np.eye(128, dtype=np.float32)
    in_maps = []
    for i in range(p.n_cores):
        xi = np.zeros((SPAD, F), np.float32)
        xi[:S] = x[i * S:(i + 1) * S]
        dinv_i = np.ones(SPAD, np.float32)
        dinv_i[:S] = p.dinv[i * S:(i + 1) * S]
        x_pm = xi.reshape(NCH, 128, F).transpose(1, 0, 2).reshape(128, NCH * F)
        dinv_col = dinv_i.reshape(NCH, 128).T.copy()  # [128, NCH]
        diag = np.zeros((128, NCH, 128), np.float32)
        rng = np.arange(128)
        diag[rng, :, rng] = dinv_col
        in_maps.append({
            "xT": np.ascontiguousarray(xi.T).astype(BF16),
            "xn": x_pm.astype(np.float32),
            "dinvc": dinv_col.astype(np.float32),
            "diag": diag.reshape(128, NCH * 128).astype(BF16),
            "lin1r": lin1.T.astype(BF16), "aw1r": aw1.T.astype(BF16),
            "lin2r": lin2.T.astype(BF16), "aw2r": aw2.T.astype(BF16),
            "b1r": b1.reshape(1, F).astype(BF16),
            "b2r": b2.reshape(1, F).astype(BF16),
            "onesr": np.ones((1, 128), BF16),
            "ident": ident,
            "idxlo": p.idx_arrs[i][0], "idxhi": p.idx_arrs[i][1],
            "dall": p.dall_arrs[i],
        })
    return in_maps


# ---------------------------------------------------------------------------
# Bass graph
# ---------------------------------------------------------------------------


def _build(p):
    import concourse.bass as bass
    import concourse.bacc as bacc
    import concourse.mybir as mybir
    import concourse.tile as tile

    dt = mybir.dt
    F = 128
    SPAD, NCH = p.SPAD, p.NCH
    HROWS, THROWS = p.HROWS, p.THROWS

    nc = bacc.Bacc("TRN2", target_bir_lowering=False, debug=False,
                   num_devices=p.n_cores, num_swdge_queues=4)

    def din(name, shape, d):
        return nc.dram_tensor(name, shape, d, kind="ExternalInput")

    xT_d = din("xT", [F, SPAD], dt.bfloat16)
    xn_d = din("xn", [128, SPAD], dt.float32)
    dinvc_d = din("dinvc", [128, NCH], dt.float32)
    diag_d = din("diag", [128, NCH * 128], dt.bfloat16)
    lin1r_d = din("lin1r", [F, F], dt.bfloat16)
    aw1r_d = din("aw1r", [F, F], dt.bfloat16)
    lin2r_d = din("lin2r", [F, F], dt.bfloat16)
    aw2r_d = din("aw2r", [F, F], dt.bfloat16)
    b1r_d = din("b1r", [1, F], dt.bfloat16)
    b2r_d = din("b2r", [1, F], dt.bfloat16)
    ones_d = din("onesr", [1, 128], dt.bfloat16)
    ident_d = din("ident", [128, 128], dt.float32)
    idxlo_d = din("idxlo", [128, p.idx_cols16[0]], dt.int16)
    idxhi_d = din("idxhi", [128, p.idx_cols16[1]], dt.int16)
    dall_d = din("dall", [128, p.DCOLS], dt.bfloat16)
    out_d = nc.dram_tensor("out", [128, SPAD], dt.float32, kind="ExternalOutput")

    AF = mybir.ActivationFunctionType
    ALU = mybir.AluOpType
    AX = mybir.AxisListType

    with tile.TileContext(nc) as tc:
        with tc.tile_pool(name="const", bufs=1) as cp, \
             tc.tile_pool(name="msg", bufs=4) as mp, \
             tc.tile_pool(name="dwin", bufs=3) as dwp, \
             tc.tile_pool(name="tmp", bufs=2) as tp, \
             tc.tile_pool(name="sc", bufs=3) as scp, \
             tc.tile_pool(name="acc", bufs=4, space="PSUM") as pacc, \
             tc.tile_pool(name="psh", bufs=2, space="PSUM") as ph, \
             tc.tile_pool(name="dram", bufs=1, space="DRAM") as dr:

            def load(d, shape, dtp, tag):
                t = cp.tile(shape, dtp, tag=tag)
                nc.sync.dma_start(t[:], d.ap())
                return t

            xT = load(xT_d, [F, SPAD], dt.bfloat16, "xT")
            xn = load(xn_d, [128, SPAD], dt.float32, "xn")
            dinvc = load(dinvc_d, [128, NCH], dt.float32, "dinvc")
            diag = load(diag_d, [128, NCH * 128], dt.bfloat16, "diag")
            lin1r = load(lin1r_d, [F, F], dt.bfloat16, "lin1r")
            aw1r = load(aw1r_d, [F, F], dt.bfloat16, "aw1r")
            lin2r = load(lin2r_d, [F, F], dt.bfloat16, "lin2r")
            aw2r = load(aw2r_d, [F, F], dt.bfloat16, "aw2r")
            b1r = load(b1r_d, [1, F], dt.bfloat16, "b1r")
            b2r = load(b2r_d, [1, F], dt.bfloat16, "b2r")
            onesr = load(ones_d, [1, 128], dt.bfloat16, "onesr")
            ident = load(ident_d, [128, 128], dt.float32, "ident")
            idx_sb = [load(idxlo_d, [128, p.idx_cols16[0]], dt.int16, "idxlo"),
                      load(idxhi_d, [128, p.idx_cols16[1]], dt.int16, "idxhi")]

            y2n = cp.tile([128, SPAD], dt.float32, tag="y2n")
            y2T = cp.tile([F, SPAD], dt.bfloat16, tag="y2T")
            outsb = None

            rg = [list(range(p.n_cores))]

            for li in range(2):
                statT = xT if li == 0 else y2T
                resid = xn if li == 0 else y2n
                linr = lin1r if li == 0 else lin2r
                awr = aw1r if li == 0 else aw2r
                br = b1r if li == 0 else b2r

                # ---- hs = (y @ lin^T) * dinv  (bf16, node-orient) ----
                hs = cp.tile([128, SPAD], dt.bfloat16, tag="hs")
                for c in range(NCH):
                    cs = slice(c * 128, (c + 1) * 128)
                    hp = ph.tile([128, F], dt.float32, tag="hp")
                    nc.tensor.matmul(hp[:], statT[:, cs], linr[:],
                                     start=True, stop=True)
                    nc.scalar.activation(hs[:, cs], hp[:], AF.Copy,
                                         scale=dinvc[:, c:c + 1])

                # ---- AllGather bf16 half-tables ----
                tables = []
                for s_ in (0, 1):
                    bounce = dr.tile([HROWS, F], dt.bfloat16,
                                     tag=f"bounce{li}_{s_}")
                    nc.sync.dma_start(
                        bounce[:, :].rearrange("(q c) f -> q (c f)", q=64),
                        hs[s_ * 64:(s_ + 1) * 64, :])
                    table = dr.tile([THROWS, F], dt.bfloat16,
                                    tag=f"table{li}_{s_}", addr_space="Shared")
                    nc.gpsimd.collective_compute(
                        "AllGather", ALU.bypass, replica_groups=rg,
                        ins=[bounce[:, :].opt()], outs=[table[:, :].opt()])
                    tables.append(table)

                if li == 1:
                    outsb = cp.tile([128, SPAD], dt.float32, tag="xn")

                # ---- message passing over windows ----
                ci = 0
                for wi, (w0, w1) in enumerate(p.windows):
                    Wc = w1 - w0
                    W = Wc * 128
                    ws = slice(w0 * 128, w1 * 128)
                    pb0, pb1 = p.wpairrange[wi]
                    d0 = int(p.dcoloff[pb0])
                    d1 = int(p.dcoloff[pb1])
                    dwin = dwp.tile([128, d1 - d0], dt.bfloat16, tag="dwin")
                    nc.sync.dma_start(dwin[:], dall_d.ap()[:, d0:d1])
                    msgs = {}
                    for s_ in (0, 1):
                        cl = p.calls[ci]; ci += 1
                        assert cl["s"] == s_ and cl["w0"] == w0
                        m = mp.tile([128, cl["G"], F], dt.bfloat16,
                                    tag=f"msg{s_}")
                        nc.gpsimd.dma_gather(
                            m[:, :, :], tables[s_][:, :],
                            idx_sb[s_][:, cl["off16"]:cl["off16"] + cl["L"] // 16],
                            num_idxs=cl["L"], num_idxs_reg=cl["L"],
                            elem_size=F, single_packet=False,
                            queue_num=(wi * 2 + s_) % 4)
                        msgs[s_] = m
                    tw = tp.tile([128, W], dt.float32, tag="tw")
                    for c in range(w0, w1):
                        cs = slice(c * 128, (c + 1) * 128)
                        acc = pacc.tile([128, F], dt.float32, tag="acc")
                        nc.tensor.matmul(acc[:], diag[:, cs], hs[:, cs],
                                         start=True, stop=False)
                        nc.tensor.matmul(acc[:], onesr[0:1, :], br[0:1, :],
                                         start=False, stop=False)
                        cps = p.chunk_pairs[c]
                        nc.tensor.matmul(acc[:], statT[:, cs], awr[:],
                                         start=False, stop=not cps)
                        for j, (s_, g, pi) in enumerate(cps):
                            lo = int(p.pair_lo[pi])
                            span = int(p.pair_span[pi])
                            loc = int(p.dcoloff[pi]) - d0
                            nc.tensor.matmul(
                                acc[lo:lo + span, :],
                                dwin[:, loc:loc + span],
                                msgs[s_][:, g, :],
                                start=False, stop=(j == len(cps) - 1))
                        nc.scalar.activation(
                            tw[:, (c - w0) * 128:(c - w0 + 1) * 128],
                            acc[:], AF.Tanh)
                    # ---- window epilogue ----
                    r01 = tp.tile([128, W], dt.float32, tag="r01")
                    nc.scalar.activation(r01[:], tw[:], AF.Copy, scale=EPS)
                    rx = tp.tile([128, W], dt.float32, tag="rx")
                    nc.vector.tensor_add(rx[:], r01[:], resid[:, ws])
                    if li == 0:
                        rr = tp.tile([128, W], dt.float32, tag="r01")
                        nc.scalar.activation(rr[:], rx[:], AF.Copy, scale=0.01)
                        x1 = tp.tile([128, W], dt.float32, tag="x1")
                        nc.vector.tensor_max(x1[:], rx[:], rr[:])
                        nc.vector.tensor_add(y2n[:, ws], x1[:], xn[:, ws])
                        for c in range(w0, w1):
                            cs = slice(c * 128, (c + 1) * 128)
                            pt = ph.tile([128, 128], dt.float32, tag="hp")
                            nc.tensor.transpose(pt[:], y2n[:, cs], ident[:])
                            nc.scalar.activation(y2T[:, cs], pt[:], AF.Copy)
                    else:
                        ex = tp.tile([128, W], dt.float32, tag="x1")
                        nc.scalar.activation(ex[:], rx[:], AF.Exp)
                        sm = scp.tile([128, Wc], dt.float32, tag="sm")
                        nc.vector.reduce_sum(
                            sm[:], ex[:].rearrange("p (c f) -> p c f", f=128),
                            axis=AX.X)
                        rc = scp.tile([128, Wc], dt.float32, tag="rc")
                        nc.vector.reciprocal(rc[:], sm[:])
                        for c in range(w0, w1):
                            cs = slice(c * 128, (c + 1) * 128)
                            j = c - w0
                            nc.scalar.activation(
                                outsb[:, cs], ex[:, j * 128:(j + 1) * 128],
                                AF.Copy, scale=rc[:, j:j + 1])

            nc.sync.dma_start(out_d.ap(), outsb[:])

    nc.compile()
    return nc


# ---------------------------------------------------------------------------
# Entry points
# ---------------------------------------------------------------------------


def _install_ntff_hook():
    """Register the axon NTFF profiling hook if the image lacks
    antenv.axon_hooks (lets run_bass_kernel_spmd(trace=True) return
    exec_time_ns + a perfetto trace)."""
    import types

    try:
        from antenv.axon_hooks import get_axon_ntff_profile_hook  # noqa: F401
        return
    except ImportError:
        pass
    try:
        import antenv

        store = {"h": None}
        mod = types.ModuleType("antenv.axon_hooks")
        mod.set_axon_ntff_profile_hook = lambda h: store.__setitem__("h", h)
        mod.get_axon_ntff_profile_hook = lambda: store["h"]
        sys.modules["antenv.axon_hooks"] = mod
        antenv.axon_hooks = mod
        if "/root/.axon_site" not in sys.path:
            sys.path.insert(0, "/root/.axon_site")
        from trn_agent_boot import trn_boot as tb

        hook = tb._ntff_profile_via_ctypes("/opt/axon/libaxon_pjrt.so")
        mod.set_axon_ntff_profile_hook(hook)
    except Exception as e:  # profiling is best-effort
        print(f"ntff hook install failed: {e}")


def _run(inputs, n_cores=8, trace=None):
    from concourse import bass_utils

    x = np.asarray(inputs["x"], np.float32)
    W1 = np.asarray(inputs["W1"], np.float32)
    b1 = np.asarray(inputs["b1"], np.float32)
    lin1 = np.asarray(inputs["lin1"], np.float32)
    W2 = np.asarray(inputs["W2"], np.float32)
    b2 = np.asarray(inputs["b2"], np.float32)
    lin2 = np.asarray(inputs["lin2"], np.float32)
    ei = np.asarray(inputs["edge_index"])

    N = x.shape[0]
    p = _host_prep(ei, N, n_cores)
    in_maps = _per_core_inputs(p, x, W1, b1, lin1, W2, b2, lin2)
    nc = _build(p)

    if trace is None:
        trace = os.environ.get("KERNEL_TRACE", "0") == "1"
    if trace:
        _install_ntff_hook()
    res = bass_utils.run_bass_kernel_spmd(
        nc, in_maps, core_ids=list(range(n_cores)), trace=trace)
    LAST_EXEC_NS["ns"] = res.exec_time_ns

    outs = []
    for i in range(n_cores):
        o = res.results[i]["out"]  # [128, SPAD] partition-major
        o = o.reshape(128, p.NCH, 128).transpose(1, 0, 2).reshape(p.SPAD, 128)
        outs.append(o[:p.S])
    return np.concatenate(outs, axis=0).astype(np.float32)


def kernel(**inputs):
    return _run(inputs)
